# revision 17
# baseline (speedup 1.0000x reference)
# DigitCaps dynamic-routing kernel for 8 Trainium2 NeuronCores (v2).
#
# Sharding: prev-layer node axis P=6272 split across 8 cores (784 each).
# Per core both W layouts are SBUF-resident: wp2 (bf16, lhsT of the s
# matmuls) and w2f8 (fp8e4, DoubleRow moving operand of the wv matmuls).
# Every routing sweep recomputes s/a from SBUF; cross-core traffic is one
# small AllReduce per (iteration, capsule-group) so the five capsule
# groups pipeline against each other across engines.
#
# Per-(g,h) tile layout glossary (local p in [0,784), PPAD=896):
#   NB layout: partitions = (ns, bs) = 8 caps x 16 batch
#   P  layout: partitions = local p (7 chunks of 128, last 16 valid)
#   wp2  [128=p][7ch][40n][128=(i,o)]   lhsT of the s matmul
#   up2  [128=p][7ch][8i][32b]          rhs of the s matmul (iter 1) / cu input
#   urep [2h][128=(ns,bs)][784p][8i]    u replicated over ns, for the a-fold
#   w2f8 [5g][64k][2ko][784p][8i]       DoubleRow moving operand, K=(ns,o)=2k+ko
#   mask2[64k][2ko][128=(ns,bs)]        block-diag v mask in DoubleRow layout
import os
import numpy as np
import ml_dtypes

import concourse.bass as bass
import concourse.bacc as bacc
import concourse.tile as tile
import concourse.mybir as mybir
from concourse.bass_utils import run_bass_kernel_spmd

BF16 = mybir.dt.bfloat16
F32 = mybir.dt.float32
FP8 = mybir.dt.float8e4
AX = mybir.AxisListType
ALU = mybir.AluOpType
ACTF = mybir.ActivationFunctionType
PERF = mybir.MatmulPerfMode

N, P, I, O, B = 40, 6272, 8, 16, 32
NC = 8
PL = P // NC          # 784 local nodes
NG = 5                # n-groups of 8
BH = 2                # b-halves of 16
CH = 7                # p-chunks of 128 (last has 16 valid rows)
PPAD = CH * 128       # 896
NB_TILES = NG * BH    # 10 (g, h) tiles; tile t = 2*g + h
GLEN = 16 * 256 + 128 * 2  # per-g collective payload: sraw + Z


def _build_program(for_sim=False):
    nc = bacc.Bacc("TRN2", target_bir_lowering=False, debug=False)

    wp2 = nc.dram_tensor("wp2", [128, CH, N, 128], BF16, kind="ExternalInput")
    up2 = nc.dram_tensor("up2", [128, CH, I, B], BF16, kind="ExternalInput")
    urep = nc.dram_tensor("urep", [BH, 128, I, PL], BF16, kind="ExternalInput")
    # partition = K = (ns,o) = ns*16+o
    w2f8 = nc.dram_tensor("w2f8", [128, NG, PL, I], FP8, kind="ExternalInput")
    mask2 = nc.dram_tensor("mask2", [128, 128], BF16, kind="ExternalInput")
    rep2 = nc.dram_tensor("rep2", [16, 128], BF16, kind="ExternalInput")
    selio = nc.dram_tensor("selio", [128, I, 16], BF16, kind="ExternalInput")
    id128b = nc.dram_tensor("id128b", [128, 128], BF16, kind="ExternalInput")
    id16f = nc.dram_tensor("id16f", [16, 16], F32, kind="ExternalInput")
    vout = nc.dram_tensor("vout", [N, B, O], F32, kind="ExternalOutput")

    with tile.TileContext(nc) as tc:
        with (
            tc.tile_pool(name="res", bufs=1) as res,
            tc.tile_pool(name="cupool", bufs=2) as cupool,
            tc.tile_pool(name="ssb", bufs=2) as ssbp,     # Ssb staging
            tc.tile_pool(name="tsp", bufs=1) as tsp,      # wv fold staging
            tc.tile_pool(name="ap", bufs=2) as apool,     # at/b2/e tiles
            tc.tile_pool(name="sm", bufs=2) as sm,        # small per-g stats
            tc.tile_pool(name="ps_s", bufs=1, space="PSUM") as ps_s,
            tc.tile_pool(name="ps_wv", bufs=1, space="PSUM") as ps_wv,
            tc.tile_pool(name="ps_m", bufs=2, space="PSUM") as ps_m,
            tc.tile_pool(name="dram", bufs=2, space="DRAM") as dram,
        ):
            # ---- residents ----
            sb_wp2 = res.tile([128, CH, N, 128], BF16)
            nc.sync.dma_start(out=sb_wp2, in_=wp2[:])
            sb_up2 = res.tile([128, CH, I, B], BF16)
            nc.sync.dma_start(out=sb_up2, in_=up2[:])
            sb_urep0 = res.tile([128, I, PL], BF16)
            nc.sync.dma_start(out=sb_urep0, in_=urep[0])
            sb_urep1 = res.tile([128, I, PL], BF16)
            nc.sync.dma_start(out=sb_urep1, in_=urep[1])
            sb_urep = [sb_urep0, sb_urep1]
            sb_w2f8 = res.tile([128, NG, PL, I], FP8)
            nc.gpsimd.dma_start(out=sb_w2f8, in_=w2f8[:])
            sb_mask2 = res.tile([128, 128], BF16)
            nc.sync.dma_start(out=sb_mask2, in_=mask2[:])
            sb_rep2 = res.tile([16, 128], BF16)
            nc.sync.dma_start(out=sb_rep2, in_=rep2[:])
            sb_sel = res.tile([128, I, 16], BF16)
            nc.sync.dma_start(out=sb_sel, in_=selio[:])
            sb_id128b = res.tile([128, 128], BF16)
            nc.sync.dma_start(out=sb_id128b, in_=id128b[:])
            sb_id16f = res.tile([16, 16], F32)
            nc.sync.dma_start(out=sb_id16f, in_=id16f[:])

            sb_eP2 = res.tile([128, CH, N, B], BF16)
            nc.vector.memset(sb_eP2, 0.0)
            sb_Z = res.tile([128, NB_TILES], F32)
            nc.vector.memset(sb_Z, float(P) / NC)
            sb_a1s = res.tile([128, NB_TILES, PL], BF16)

            def s_block(it, g):
                """Partial s_raw for the 8 capsules of group g; returns the
                SBUF staging tile [16, 8n, 32b] holding local partial sums."""
                # columns ordered (h, ns, bs): col 128*h + 16*ns + bs
                sraw = sm.tile([16, 2, 8, 16], F32, tag="sraw")
                for q in range(2):  # two 4-capsule blocks
                    psum_s = ps_s.tile([128, 4, 256], F32, tag="ps_s", bufs=1)
                    for nn in range(4):
                        n = 8 * g + 4 * q + nn
                        if it == 1:
                            cu = sb_up2
                        else:
                            cu = cupool.tile([128, CH, I, B], BF16, tag="cu")
                            e_sl = bass.AP(
                                tensor=sb_eP2.tensor,
                                offset=sb_eP2.offset + n * B,
                                ap=[sb_eP2.ap[0], [N * B, CH], [0, I], [1, B]],
                            )
                            nc.vector.tensor_tensor(
                                out=cu, in0=sb_up2, in1=e_sl, op=ALU.mult
                            )
                        cu_flat = cu.rearrange("q c i b -> q c (i b)")
                        for ch in range(CH):
                            nc.tensor.matmul(
                                psum_s[:, nn, :],
                                sb_wp2[:, ch, n, :],
                                cu_flat[:, ch, :],
                                start=(ch == 0),
                                stop=(ch == CH - 1),
                            )
                    # extract s[o,(n,b)] = sum_i psum[(i,o), nn, (i,b)]
                    Ssb = ssbp.tile([128, I, 4, B], BF16, tag="S2")
                    nc.scalar.copy(
                        out=Ssb.rearrange("q i n b -> q n i b"),
                        in_=psum_s.rearrange("q n (i b) -> q n i b", i=I),
                    )
                    Ssb_flat = Ssb.rearrange("q i n b -> q i (n b)")
                    sel_ps = ps_m.tile([16, 4, B], F32, tag="m", bufs=1)
                    for i in range(I):
                        nc.tensor.matmul(
                            sel_ps,
                            sb_sel[:, i, :],
                            Ssb_flat[:, i, :],
                            start=(i == 0),
                            stop=(i == I - 1),
                        )
                    nc.scalar.copy(
                        out=sraw[:, :, 4 * q : 4 * q + 4, :],
                        in_=sel_ps.rearrange("o n (h b) -> o h n b", h=BH),
                    )
                return sraw

            def cc_block(it, g, sraw):
                """Per-g AllReduce of (sraw ++ Z columns 2g,2g+1)."""
                cc_in = dram.tile([GLEN], F32, tag="cc_in")
                cc_out = dram.tile([GLEN], F32, tag="cc_out")
                nc.sync.dma_start(
                    out=cc_in[0 : 16 * 256],
                    in_=sraw.rearrange("o h n b -> o (h n b)"),
                )
                nc.sync.dma_start(
                    out=cc_in[16 * 256 :], in_=sb_Z[:, 2 * g : 2 * g + 2]
                )
                if for_sim:
                    nc.gpsimd.dma_start(out=cc_out, in_=cc_in)
                else:
                    nc.gpsimd.collective_compute(
                        "AllReduce",
                        ALU.add,
                        replica_groups=[list(range(NC))],
                        ins=[cc_in.opt()],
                        outs=[cc_out.opt()],
                    )
                sglob = sm.tile([16, 256], F32, tag="sglob")
                Zg = sm.tile([128, 2], F32, tag="zg")
                nc.sync.dma_start(out=sglob, in_=cc_out[0 : 16 * 256])
                nc.sync.dma_start(out=Zg, in_=cc_out[16 * 256 :])
                return sglob, Zg

            def squash_block(it, g, sglob, Zg):
                """Squash for both h tiles of g. it<3: returns (bd fp8 tiles,
                fac). it==3: writes vout."""
                ss = sm.tile([128, 2], F32, tag="ss")
                sq_sb = []
                for h in range(BH):
                    sp = ps_m.tile([128, 16], F32, tag="sq", bufs=1)
                    nc.tensor.transpose(
                        sp, sglob[:, 128 * h : 128 * (h + 1)], sb_id16f
                    )
                    sq = sm.tile([128, 16], F32, tag="sqs", bufs=2)
                    nc.vector.tensor_copy(out=sq, in_=sp)
                    sq2 = sm.tile([128, 16], F32, tag="sq2", bufs=1)
                    nc.vector.tensor_tensor(out=sq2, in0=sq, in1=sq, op=ALU.mult)
                    nc.vector.tensor_reduce(
                        out=ss[:, h : h + 1], in_=sq2, axis=AX.X, op=ALU.add
                    )
                    sq_sb.append(sq)
                z2 = sm.tile([128, 2], F32, tag="z2", bufs=1)
                nc.vector.tensor_tensor(out=z2, in0=Zg, in1=Zg, op=ALU.mult)
                den = sm.tile([128, 2], F32, tag="den", bufs=1)
                nc.vector.tensor_tensor(out=den, in0=z2, in1=ss, op=ALU.add)
                rec = sm.tile([128, 2], F32, tag="rec")
                nc.vector.reciprocal(out=rec, in_=den)
                rss = sm.tile([128, 2], F32, tag="rss")
                nc.scalar.sqrt(out=rss, in_=ss)
                fac = sm.tile([128, 2], F32, tag="fac")
                nc.vector.tensor_tensor(out=fac, in0=rss, in1=rec, op=ALU.mult)

                if it == 3:
                    for h in range(BH):
                        vt = sm.tile([128, 16], F32, tag="vt", bufs=1)
                        nc.vector.tensor_scalar_mul(
                            vt, sq_sb[h], fac[:, h : h + 1]
                        )
                        nc.sync.dma_start(
                            out=vout[8 * g : 8 * g + 8, 16 * h : 16 * h + 16, :],
                            in_=vt,
                        )
                    return None, None

                # v^T/4096 for the wv matmuls (keeps fp8 bd in range;
                # the 4096 is folded back into fac4 below)
                fac4 = sm.tile([128, 2], F32, tag="fac4")
                nc.vector.tensor_scalar_mul(fac4, fac, 4096.0)
                vT = sm.tile([16, 256], BF16, tag="vT", bufs=1)
                nc.scalar.mul(out=vT, in_=sglob, mul=1.0 / 4096.0)
                v8_ps = ps_m.tile([128, 256], F32, tag="v8", bufs=1)
                nc.tensor.matmul(v8_ps, sb_rep2, vT, start=True, stop=True)
                bds = []
                for h in range(BH):
                    bd = sm.tile([128, 128], BF16, tag="bd", bufs=2)
                    nc.vector.tensor_tensor(
                        out=bd,
                        in0=v8_ps[:, 128 * h : 128 * (h + 1)],
                        in1=sb_mask2,
                        op=ALU.mult,
                    )
                    bds.append(bd)
                return bds, fac4

            def a_block(it, g, bds, fac):
                """a = (u_hat . v)*fac for both h tiles of g, then the bridge
                into the next s-pass (exp with Z accum, transpose into eP2)."""
                for h in range(BH):
                    t_ = 2 * g + h
                    ts = tsp.tile([128, I, PL], BF16, tag="ts")
                    # 13 DoubleRow wv matmuls (12x64p + 1x16p), psum-grouped 2
                    off = 0
                    for grp in range(7):
                        pws = [64, 64] if grp < 6 else [16]
                        wv_ps = ps_wv.tile([128, 2, 512], F32, tag="wv", bufs=1)
                        for j, pw in enumerate(pws):
                            F = pw * I
                            nc.tensor.matmul(
                                wv_ps[:, j, :F],
                                bds[h],
                                sb_w2f8[:, g, off : off + pw, :]
                                .rearrange("k p i -> k (p i)"),
                                start=True,
                                stop=True,
                            )
                            off += pw
                        gw = sum(pws)
                        # strided i-major evacuation: ts[i, p] = psum[(p,i)]
                        nc.scalar.copy(
                            out=ts[:, :, off - gw : off]
                            .rearrange("q i p -> q p i"),
                            in_=wv_ps.rearrange("q c (p i) -> q (c p) i", i=I)[
                                :, 0:gw, :
                            ],
                        )
                    # fold: mult by u then reduce over i (all in place, 2x)
                    nc.vector.tensor_tensor(
                        out=ts, in0=ts, in1=sb_urep[h], op=ALU.mult
                    )
                    nc.vector.tensor_tensor(
                        out=ts[:, 0:4, :], in0=ts[:, 0:4, :], in1=ts[:, 4:8, :],
                        op=ALU.add,
                    )
                    nc.vector.tensor_tensor(
                        out=ts[:, 0:2, :], in0=ts[:, 0:2, :], in1=ts[:, 2:4, :],
                        op=ALU.add,
                    )
                    at = apool.tile([128, PL], BF16, tag="at")
                    nc.vector.tensor_tensor(
                        out=at, in0=ts[:, 0, :], in1=ts[:, 1, :], op=ALU.add
                    )
                    # b = fac*a (+ previous scaled a on iter 2)
                    b2 = apool.tile([128, PL], BF16, tag="b2")
                    nc.vector.tensor_scalar_mul(b2, at, fac[:, h : h + 1])
                    if it == 1:
                        nc.vector.tensor_copy(out=sb_a1s[:, t_, :], in_=b2)
                    else:
                        nc.vector.tensor_tensor(
                            out=b2, in0=b2, in1=sb_a1s[:, t_, :], op=ALU.add
                        )
                    bt = b2
                    e_nb = apool.tile([128, PPAD], BF16, tag="enb")
                    nc.vector.memset(e_nb[:, PL:], 1.0)
                    nc.scalar.activation(
                        out=e_nb[:, 0:PL], in_=bt, func=ACTF.Exp,
                        accum_out=sb_Z[:, t_ : t_ + 1],
                    )
                    eT_ps = ps_wv.tile([128, CH, 128], BF16, tag="eT", bufs=1)
                    for ch in range(CH):
                        nc.tensor.transpose(
                            eT_ps[:, ch, :],
                            e_nb[:, 128 * ch : 128 * (ch + 1)],
                            sb_id128b,
                        )
                    nc.vector.tensor_copy(
                        out=sb_eP2[:, :, 8 * g : 8 * g + 8,
                                   16 * h : 16 * h + 16],
                        in_=eT_ps.rearrange("p c (n b) -> p c n b", n=8),
                    )  # DVE: Pool has no PSUM access

            for it in (1, 2, 3):
                ccs = []
                for g in range(NG):
                    sraw = s_block(it, g)
                    ccs.append(cc_block(it, g, sraw))
                for g in range(NG):
                    sglob, Zg = ccs[g]
                    bds, fac = squash_block(it, g, sglob, Zg)
                    if it < 3:
                        a_block(it, g, bds, fac)

    nc.finalize()
    return nc


_CACHE = {}


def _prep_inputs(u, W):
    """Per-core host-side relayout (not part of HW time)."""
    bf = ml_dtypes.bfloat16
    f8 = ml_dtypes.float8_e4m3
    maps = []
    # constant tensors shared by all cores
    mask2 = np.zeros((128, 128), np.float32)
    rep2 = np.zeros((16, 128), np.float32)
    for m in range(128):
        ns, o = m // 16, m % 16
        mask2[m, ns * 16 : ns * 16 + 16] = 1.0
        rep2[o, m] = 1.0
    sel = np.zeros((128, I, 16), np.float32)
    for i in range(I):
        sel[16 * i : 16 * i + 16, i] = np.eye(16, dtype=np.float32)
    id128 = np.eye(128, dtype=np.float32)

    for c in range(NC):
        sl = slice(PL * c, PL * (c + 1))
        Wc = np.ascontiguousarray(W[:, sl])          # [40, 784, 8, 16] f32
        uc = np.ascontiguousarray(u[:, sl])          # [32, 784, 8] f32
        Wp = np.zeros((N, PPAD, I, O), np.float32)
        Wp[:, :PL] = Wc
        wp2 = Wp.reshape(N, CH, 128, 128).transpose(2, 1, 0, 3)
        Up = np.zeros((B, PPAD, I), np.float32)
        Up[:, :PL] = uc
        up2 = Up.reshape(B, CH, 128, I).transpose(2, 1, 3, 0)
        ur = np.broadcast_to(
            uc.transpose(0, 2, 1).reshape(1, BH, 16, I, PL),
            (8, BH, 16, I, PL),
        ).transpose(1, 0, 2, 3, 4).reshape(BH, 128, I, PL)
        # w2f8[K=(ns*16+o), g, p, i] = W[8g+ns, p, i, o]
        w2 = (
            Wc.reshape(NG, 8, PL, I, O)
            .transpose(0, 1, 4, 2, 3)                # [g, ns, o, p, i]
            .reshape(NG, 128, PL, I)
            .transpose(1, 0, 2, 3)                   # [K, g, p, i]
        )
        w2 = np.ascontiguousarray(w2)
        maps.append(
            {
                "wp2": np.ascontiguousarray(wp2).astype(bf),
                "up2": np.ascontiguousarray(up2).astype(bf),
                "urep": np.ascontiguousarray(ur).astype(bf),
                "w2f8": np.ascontiguousarray(w2).astype(f8),
                "mask2": mask2.astype(bf),
                "rep2": rep2.astype(bf),
                "selio": sel.astype(bf),
                "id128b": id128.astype(bf),
                "id16f": np.eye(16, dtype=np.float32),
            }
        )
    return maps


def kernel(u, W):
    u = np.asarray(u, np.float32)
    W = np.asarray(W, np.float32)
    if "nc" not in _CACHE:
        _CACHE["nc"] = _build_program()
    nc = _CACHE["nc"]
    in_maps = _prep_inputs(u, W)
    res = run_bass_kernel_spmd(
        nc, in_maps, core_ids=list(range(NC)),
        trace=bool(int(os.environ.get("KERNEL_TRACE", "0"))),
    )
    _CACHE["last_result"] = res
    return res.results[0]["vout"]


# revision 27
# speedup vs baseline: 1.3240x; 1.3240x over previous
# DigitCaps dynamic-routing kernel for 8 Trainium2 NeuronCores (v2).
#
# Sharding: prev-layer node axis P=6272 split across 8 cores (784 each).
# Per core both W layouts are SBUF-resident: wp2 (bf16, lhsT of the s
# matmuls) and w2f8 (fp8e4, DoubleRow moving operand of the wv matmuls).
# Every routing sweep recomputes s/a from SBUF; cross-core traffic is one
# small AllReduce per (iteration, capsule-group) so the five capsule
# groups pipeline against each other across engines.
#
# Per-(g,h) tile layout glossary (local p in [0,784), PPAD=896):
#   NB layout: partitions = (ns, bs) = 8 caps x 16 batch
#   P  layout: partitions = local p (7 chunks of 128, last 16 valid)
#   wp2  [128=p][7ch][40n][128=(i,o)]   lhsT of the s matmul
#   up2  [128=p][7ch][8i][32b]          rhs of the s matmul (iter 1) / cu input
#   urep [2h][128=(ns,bs)][784p][8i]    u replicated over ns, for the a-fold
#   w2f8 [5g][64k][2ko][784p][8i]       DoubleRow moving operand, K=(ns,o)=2k+ko
#   mask2[64k][2ko][128=(ns,bs)]        block-diag v mask in DoubleRow layout
import os
import numpy as np
import ml_dtypes

import concourse.bass as bass
import concourse.bacc as bacc
import concourse.tile as tile
import concourse.mybir as mybir
from concourse.bass_utils import run_bass_kernel_spmd

BF16 = mybir.dt.bfloat16
F32 = mybir.dt.float32
FP8 = mybir.dt.float8e4
AX = mybir.AxisListType
ALU = mybir.AluOpType
ACTF = mybir.ActivationFunctionType
PERF = mybir.MatmulPerfMode

N, P, I, O, B = 40, 6272, 8, 16, 32
NC = 8
PL = P // NC          # 784 local nodes
NG = 5                # n-groups of 8
BH = 2                # b-halves of 16
CH = 7                # p-chunks of 128 (last has 16 valid rows)
PPAD = CH * 128       # 896
NB_TILES = NG * BH    # 10 (g, h) tiles; tile t = 2*g + h
GLEN = 16 * 256 + 128 * 2  # per-g collective payload: sraw + Z


def _build_program(for_sim=False):
    nc = bacc.Bacc("TRN2", target_bir_lowering=False, debug=False)

    wp2 = nc.dram_tensor("wp2", [128, CH, N, 128], BF16, kind="ExternalInput")
    up2 = nc.dram_tensor("up2", [128, CH, I, B], BF16, kind="ExternalInput")
    urep = nc.dram_tensor("urep", [BH, 128, I, PL], BF16, kind="ExternalInput")
    # partition = K = (ns,o) = ns*16+o
    w2f8 = nc.dram_tensor("w2f8", [128, NG, PL, I], FP8, kind="ExternalInput")
    mask2 = nc.dram_tensor("mask2", [128, 128], BF16, kind="ExternalInput")
    rep2 = nc.dram_tensor("rep2", [16, 128], BF16, kind="ExternalInput")
    selio = nc.dram_tensor("selio", [128, I, 16], BF16, kind="ExternalInput")
    id128b = nc.dram_tensor("id128b", [128, 128], BF16, kind="ExternalInput")
    id16f = nc.dram_tensor("id16f", [16, 16], F32, kind="ExternalInput")
    vout = nc.dram_tensor("vout", [N, B, O], F32, kind="ExternalOutput")

    with tile.TileContext(nc) as tc:
        with (
            tc.tile_pool(name="res", bufs=1) as res,
            tc.tile_pool(name="cupool", bufs=2) as cupool,
            tc.tile_pool(name="ssb", bufs=2) as ssbp,     # Ssb staging
            tc.tile_pool(name="tsp", bufs=1) as tsp,      # wv fold staging
            tc.tile_pool(name="ap", bufs=2) as apool,     # at/b2/e tiles
            tc.tile_pool(name="sm", bufs=2) as sm,        # small per-g stats
            tc.tile_pool(name="ps_s", bufs=1, space="PSUM") as ps_s,
            tc.tile_pool(name="ps_wv", bufs=1, space="PSUM") as ps_wv,
            tc.tile_pool(name="ps_m", bufs=2, space="PSUM") as ps_m,
            tc.tile_pool(name="dram", bufs=2, space="DRAM") as dram,
        ):
            # ---- residents ----
            sb_wp2 = res.tile([128, CH, N, 128], BF16)
            nc.sync.dma_start(out=sb_wp2, in_=wp2[:])
            sb_up2 = res.tile([128, CH, I, B], BF16)
            nc.sync.dma_start(out=sb_up2, in_=up2[:])
            sb_urep0 = res.tile([128, I, PL], BF16)
            nc.sync.dma_start(out=sb_urep0, in_=urep[0])
            sb_urep1 = res.tile([128, I, PL], BF16)
            nc.sync.dma_start(out=sb_urep1, in_=urep[1])
            sb_urep = [sb_urep0, sb_urep1]
            sb_w2f8 = res.tile([128, NG, PL, I], FP8)
            nc.gpsimd.dma_start(out=sb_w2f8, in_=w2f8[:])
            sb_mask2 = res.tile([128, 128], BF16)
            nc.sync.dma_start(out=sb_mask2, in_=mask2[:])
            sb_rep2 = res.tile([16, 128], BF16)
            nc.sync.dma_start(out=sb_rep2, in_=rep2[:])
            sb_sel = res.tile([128, I, 16], BF16)
            nc.sync.dma_start(out=sb_sel, in_=selio[:])
            sb_id128b = res.tile([128, 128], BF16)
            nc.sync.dma_start(out=sb_id128b, in_=id128b[:])
            sb_id16f = res.tile([16, 16], F32)
            nc.sync.dma_start(out=sb_id16f, in_=id16f[:])

            sb_eP2 = res.tile([128, CH, N, B], BF16)
            nc.vector.memset(sb_eP2, 0.0)
            sb_Z = res.tile([128, NB_TILES], F32)
            nc.vector.memset(sb_Z, float(P) / NC)
            sb_a1s = res.tile([128, NB_TILES, PL], BF16)

            def s_block(it, g):
                """Partial s_raw for the 8 capsules of group g; returns the
                SBUF staging tile [16, 8n, 32b] holding local partial sums."""
                # columns ordered (h, ns, bs): col 128*h + 16*ns + bs
                sraw = sm.tile([16, 2, 8, 16], F32, tag="sraw")
                for q in range(2):  # two 4-capsule blocks
                    psum_s = ps_s.tile([128, 4, 256], F32, tag="ps_s", bufs=1)
                    for nn in range(4):
                        n = 8 * g + 4 * q + nn
                        if it == 1:
                            cu = sb_up2
                        else:
                            cu = cupool.tile([128, CH, I, B], BF16, tag="cu")
                            e_sl = bass.AP(
                                tensor=sb_eP2.tensor,
                                offset=sb_eP2.offset + n * B,
                                ap=[sb_eP2.ap[0], [N * B, CH], [0, I], [1, B]],
                            )
                            nc.vector.tensor_tensor(
                                out=cu, in0=sb_up2, in1=e_sl, op=ALU.mult
                            )
                        cu_flat = cu.rearrange("q c i b -> q c (i b)")
                        for ch in range(CH):
                            nc.tensor.matmul(
                                psum_s[:, nn, :],
                                sb_wp2[:, ch, n, :],
                                cu_flat[:, ch, :],
                                start=(ch == 0),
                                stop=(ch == CH - 1),
                            )
                    # extract s[o,(n,b)] = sum_i psum[(i,o), nn, (i,b)]
                    Ssb = ssbp.tile([128, I, 4, B], BF16, tag="S2")
                    nc.scalar.copy(
                        out=Ssb.rearrange("q i n b -> q n i b"),
                        in_=psum_s.rearrange("q n (i b) -> q n i b", i=I),
                    )
                    Ssb_flat = Ssb.rearrange("q i n b -> q i (n b)")
                    sel_ps = ps_m.tile([16, 4, B], F32, tag="m", bufs=1)
                    for i in range(I):
                        nc.tensor.matmul(
                            sel_ps,
                            sb_sel[:, i, :],
                            Ssb_flat[:, i, :],
                            start=(i == 0),
                            stop=(i == I - 1),
                        )
                    nc.scalar.copy(
                        out=sraw[:, :, 4 * q : 4 * q + 4, :],
                        in_=sel_ps.rearrange("o n (h b) -> o h n b", h=BH),
                    )
                return sraw

            def cc_start(it, g, sraw):
                """Per-g AllReduce of (sraw ++ Z columns 2g,2g+1): input
                DMAs + the collective. Output DMAs deferred to cc_finish."""
                cc_in = dram.tile([GLEN], F32, tag="cc_in", bufs=3)
                cc_out = dram.tile([GLEN], F32, tag="cc_out", bufs=3)
                nc.sync.dma_start(
                    out=cc_in[0 : 16 * 256],
                    in_=sraw.rearrange("o h n b -> o (h n b)"),
                )
                nc.sync.dma_start(
                    out=cc_in[16 * 256 :], in_=sb_Z[:, 2 * g : 2 * g + 2]
                )
                if for_sim:
                    nc.gpsimd.dma_start(out=cc_out, in_=cc_in)
                else:
                    nc.gpsimd.collective_compute(
                        "AllReduce",
                        ALU.add,
                        replica_groups=[list(range(NC))],
                        ins=[cc_in.opt()],
                        outs=[cc_out.opt()],
                    )
                return cc_out

            def cc_finish(it, g, cc_out):
                sglob = sm.tile([16, 256], F32, tag="sglob")
                Zg = sm.tile([128, 2], F32, tag="zg")
                nc.sync.dma_start(out=sglob, in_=cc_out[0 : 16 * 256])
                nc.sync.dma_start(out=Zg, in_=cc_out[16 * 256 :])
                return sglob, Zg

            def squash_block(it, g, sglob, Zg):
                """Squash for both h tiles of g. it<3: returns (bd fp8 tiles,
                fac). it==3: writes vout."""
                ss = sm.tile([128, 2], F32, tag="ss")
                sq_sb = []
                for h in range(BH):
                    sp = ps_m.tile([128, 16], F32, tag="sqv", bufs=1, name="sq")
                    nc.tensor.transpose(
                        sp, sglob[:, 128 * h : 128 * (h + 1)], sb_id16f
                    )
                    sq = sm.tile([128, 16], F32, tag="sqs", bufs=2)
                    nc.vector.tensor_copy(out=sq, in_=sp)
                    sq2 = sm.tile([128, 16], F32, tag="sq2", bufs=1)
                    nc.vector.tensor_tensor(out=sq2, in0=sq, in1=sq, op=ALU.mult)
                    nc.vector.tensor_reduce(
                        out=ss[:, h : h + 1], in_=sq2, axis=AX.X, op=ALU.add
                    )
                    sq_sb.append(sq)
                z2 = sm.tile([128, 2], F32, tag="z2", bufs=1)
                nc.vector.tensor_tensor(out=z2, in0=Zg, in1=Zg, op=ALU.mult)
                den = sm.tile([128, 2], F32, tag="den", bufs=1)
                nc.vector.tensor_tensor(out=den, in0=z2, in1=ss, op=ALU.add)
                rec = sm.tile([128, 2], F32, tag="rec")
                nc.vector.reciprocal(out=rec, in_=den)
                rss = sm.tile([128, 2], F32, tag="rss")
                if it == 3:
                    nc.scalar.sqrt(out=rss, in_=ss)
                else:
                    # rss = sqrt(2^24 * ss) = 4096*sqrt(ss): folds the v/4096
                    # prescale back into fac in one op
                    nc.scalar.activation(
                        out=rss, in_=ss, func=ACTF.Sqrt, scale=16777216.0
                    )
                fac = sm.tile([128, 2], F32, tag="fac")
                nc.vector.tensor_tensor(out=fac, in0=rss, in1=rec, op=ALU.mult)

                if it == 3:
                    for h in range(BH):
                        vt = sm.tile([128, 16], F32, tag="vt", bufs=1)
                        nc.vector.tensor_scalar_mul(
                            vt, sq_sb[h], fac[:, h : h + 1]
                        )
                        nc.sync.dma_start(
                            out=vout[8 * g : 8 * g + 8, 16 * h : 16 * h + 16, :],
                            in_=vt,
                        )
                    return None, None

                # v^T/4096 for the wv matmuls (bf16-safe scaling; the 4096
                # is already folded into fac via the sqrt scale)
                vT = sm.tile([16, 256], BF16, tag="vT", bufs=1)
                nc.scalar.mul(out=vT, in_=sglob, mul=1.0 / 4096.0)
                v8_ps = ps_m.tile([128, 256], F32, tag="sqv", bufs=1, name="v8")
                nc.tensor.matmul(v8_ps, sb_rep2, vT, start=True, stop=True)
                bds = []
                for h in range(BH):
                    bd = sm.tile([128, 128], BF16, tag="bd", bufs=2)
                    nc.vector.tensor_tensor(
                        out=bd,
                        in0=v8_ps[:, 128 * h : 128 * (h + 1)],
                        in1=sb_mask2,
                        op=ALU.mult,
                    )
                    bds.append(bd)
                return bds, fac

            def a_block(it, g, bds, fac):
                """a = (u_hat . v)*fac for both h tiles of g, then the bridge
                into the next s-pass (exp with Z accum, transpose into eP2)."""
                for h in range(BH):
                    t_ = 2 * g + h
                    at = apool.tile([128, PL], BF16, tag="at")
                    # 13 wv matmuls (12x64p + 1x16p) in two p-halves so the
                    # fold of one half overlaps the evacuation of the next
                    for half in range(2):
                        pb = 392 * half
                        hw_ = 392
                        ts = tsp.tile([128, I, 392], BF16, tag="ts", bufs=2)
                        off = 0
                        for grp in range(4):
                            pws = [64, 64] if grp < 3 else [8]
                            wv_ps = ps_wv.tile(
                                [128, 2, 512], F32, tag="wv", bufs=2
                            )
                            for j, pw in enumerate(pws):
                                F = pw * I
                                nc.tensor.matmul(
                                    wv_ps[:, j, :F],
                                    bds[h],
                                    sb_w2f8[:, g, pb + off : pb + off + pw, :]
                                    .rearrange("k p i -> k (p i)"),
                                    start=True,
                                    stop=True,
                                )
                                off += pw
                            gw = sum(pws)
                            nc.scalar.copy(
                                out=ts[:, :, off - gw : off]
                                .rearrange("q i p -> q p i"),
                                in_=wv_ps.rearrange(
                                    "q c (p i) -> q (c p) i", i=I
                                )[:, 0:gw, :],
                            )
                        nc.vector.tensor_tensor(
                            out=ts, in0=ts,
                            in1=sb_urep[h][:, :, pb : pb + hw_], op=ALU.mult
                        )
                        nc.vector.tensor_tensor(
                            out=ts[:, 0:4, :], in0=ts[:, 0:4, :],
                            in1=ts[:, 4:8, :], op=ALU.add,
                        )
                        nc.vector.tensor_tensor(
                            out=ts[:, 0:2, :], in0=ts[:, 0:2, :],
                            in1=ts[:, 2:4, :], op=ALU.add,
                        )
                        nc.vector.tensor_tensor(
                            out=at[:, pb : pb + hw_], in0=ts[:, 0, :],
                            in1=ts[:, 1, :], op=ALU.add,
                        )
                    # b = fac*a (+ previous scaled a on iter 2)
                    if it == 1:
                        b2 = sb_a1s[:, t_, :]
                        nc.vector.tensor_scalar_mul(b2, at, fac[:, h : h + 1])
                    else:
                        b2 = apool.tile([128, PL], BF16, tag="b2")
                        nc.vector.tensor_scalar_mul(b2, at, fac[:, h : h + 1])
                        nc.vector.tensor_tensor(
                            out=b2, in0=b2, in1=sb_a1s[:, t_, :], op=ALU.add
                        )
                    bt = b2
                    e_nb = apool.tile([128, PPAD], BF16, tag="enb")
                    if it == 1:
                        nc.vector.memset(e_nb[:, PL:], 1.0)
                    nc.scalar.activation(
                        out=e_nb[:, 0:PL], in_=bt, func=ACTF.Exp,
                        accum_out=sb_Z[:, t_ : t_ + 1],
                    )
                    eT_ps = ps_m.tile([128, CH, 128], BF16, tag="m", bufs=1, name="eT")
                    for ch in range(CH):
                        nc.tensor.transpose(
                            eT_ps[:, ch, :],
                            e_nb[:, 128 * ch : 128 * (ch + 1)],
                            sb_id128b,
                        )
                    nc.vector.tensor_copy(
                        out=sb_eP2[:, :, 8 * g : 8 * g + 8,
                                   16 * h : 16 * h + 16],
                        in_=eT_ps.rearrange("p c (n b) -> p c n b", n=8),
                    )  # DVE: Pool has no PSUM access

            for it in (1, 2, 3):
                ccs = []
                for g in range(NG):
                    sraw = s_block(it, g)
                    ccs.append(cc_start(it, g, sraw))
                for g in range(NG):
                    sglob, Zg = cc_finish(it, g, ccs[g])
                    bds, fac = squash_block(it, g, sglob, Zg)
                    if it < 3:
                        a_block(it, g, bds, fac)

    nc.finalize()
    return nc


_CACHE = {}


def _prep_inputs(u, W):
    """Per-core host-side relayout (not part of HW time)."""
    bf = ml_dtypes.bfloat16
    f8 = ml_dtypes.float8_e4m3
    maps = []
    # constant tensors shared by all cores
    mask2 = np.zeros((128, 128), np.float32)
    rep2 = np.zeros((16, 128), np.float32)
    for m in range(128):
        ns, o = m // 16, m % 16
        mask2[m, ns * 16 : ns * 16 + 16] = 1.0
        rep2[o, m] = 1.0
    sel = np.zeros((128, I, 16), np.float32)
    for i in range(I):
        sel[16 * i : 16 * i + 16, i] = np.eye(16, dtype=np.float32)
    id128 = np.eye(128, dtype=np.float32)

    for c in range(NC):
        sl = slice(PL * c, PL * (c + 1))
        Wc = np.ascontiguousarray(W[:, sl])          # [40, 784, 8, 16] f32
        uc = np.ascontiguousarray(u[:, sl])          # [32, 784, 8] f32
        Wp = np.zeros((N, PPAD, I, O), np.float32)
        Wp[:, :PL] = Wc
        wp2 = Wp.reshape(N, CH, 128, 128).transpose(2, 1, 0, 3)
        Up = np.zeros((B, PPAD, I), np.float32)
        Up[:, :PL] = uc
        up2 = Up.reshape(B, CH, 128, I).transpose(2, 1, 3, 0)
        ur = np.broadcast_to(
            uc.transpose(0, 2, 1).reshape(1, BH, 16, I, PL),
            (8, BH, 16, I, PL),
        ).transpose(1, 0, 2, 3, 4).reshape(BH, 128, I, PL)
        # w2f8[K=(ns*16+o), g, p, i] = W[8g+ns, p, i, o]
        w2 = (
            Wc.reshape(NG, 8, PL, I, O)
            .transpose(0, 1, 4, 2, 3)                # [g, ns, o, p, i]
            .reshape(NG, 128, PL, I)
            .transpose(1, 0, 2, 3)                   # [K, g, p, i]
        )
        w2 = np.ascontiguousarray(w2)
        maps.append(
            {
                "wp2": np.ascontiguousarray(wp2).astype(bf),
                "up2": np.ascontiguousarray(up2).astype(bf),
                "urep": np.ascontiguousarray(ur).astype(bf),
                "w2f8": np.ascontiguousarray(w2).astype(f8),
                "mask2": mask2.astype(bf),
                "rep2": rep2.astype(bf),
                "selio": sel.astype(bf),
                "id128b": id128.astype(bf),
                "id16f": np.eye(16, dtype=np.float32),
            }
        )
    return maps


def kernel(u, W):
    u = np.asarray(u, np.float32)
    W = np.asarray(W, np.float32)
    if "nc" not in _CACHE:
        _CACHE["nc"] = _build_program()
    nc = _CACHE["nc"]
    in_maps = _prep_inputs(u, W)
    res = run_bass_kernel_spmd(
        nc, in_maps, core_ids=list(range(NC)),
        trace=bool(int(os.environ.get("KERNEL_TRACE", "0"))),
    )
    _CACHE["last_result"] = res
    return res.results[0]["vout"]
